# revision 18
# baseline (speedup 1.0000x reference)
"""RNN-T joint network kernel for Trainium2 (8 NeuronCores, data-parallel over B).

Computes logits = relu(f @ W1f.T + g @ W1g.T + b1) @ W2.T + b2 over the
(B, T, U, ...) broadcast grid without materializing the concat tensor.

v2 strategy (per core, one batch element b) — fp8 DoubleRow main matmul:
  - The dominant cost is h @ W2.T (10.6 GMAC/core).  fp16 runs it at
    ~1 col/cycle/128-chunk on the PE; fp8e4 (e4m3) with
    MatmulPerfMode.DoubleRow contracts 256 K per column-cycle => 2x.
  - Direct e4m3 quantization of h fails the 2e-2 gate (0.044).  Fix:
    subtract a rank-structured mean field before quantizing.  With
    x = pf[t,j] + pg[u,j] (+b1) and q = relu(x):
        s2[j,t,u] = q - mt[j,t] - cu[j,u]
    where mt[j,t] ~ E_u[q] and cu[j,u] ~ E_t[q] - E[q] are the two-way
    ANOVA main effects, computed in closed form on-device:
    pf[.,j]/pg[.,j] are (exactly) Gaussian across t/u, so
    E[relu(z + sigma Z)] = sigma*(phi(zh) + zh*Phi(zh)) = sigma*(0.3989*
    exp(-zh^2/2) + Gelu(zh)) — ScalarE Gelu/Exp + tiny DVE ops.
    s2 has ~3x smaller magnitude than q, so e4m3(s2) passes easily
    (measured end-to-end rel err ~0.012 vs gate 2e-2).
  - s-gen is fully fused per (jc, u):  a = max(pf + (pg-cu), -cu)
    (= relu(x) - cu, one DVE tensor_scalar with two per-partition AP
    scalars), then s8 = e4m3(a - mt) (one tensor_tensor, fp8 out).
  - Scales: lambda_s=16 folded into W1/b1 on host, lambda_w=16 into the
    e4m3 W2 copy => PSUM = 256 * (s2@W2) and the int8 output scale is
    exactly 256: the drain is a bare fp32->int8 cast (ScalarE Identity
    activation for 7 vocab chunks, DVE tensor_copy for 1).
  - The rank images mt@W2 (V x T) and cu@W2 (V x U) are computed on
    device with tiny fp16 matmuls and DMA'd out; the host adds them
    (plus b2) during the existing dequant/transpose pass.
  - Grid u-major: g = u*T + t; spans of 10 u-rows (2000 cols, 4 PSUM
    banks of 500); the single-u tail span (u=100) is processed first to
    prime the pipeline.
"""

import sys

sys.path.insert(0, "/opt/trn_rl_repo")

import ml_dtypes
import numpy as np

from concourse import bacc, bass, tile, mybir
from concourse.bass_utils import run_bass_kernel_spmd

B, T, U = 8, 200, 101
ENC_H, PRED_H, JH, V = 1024, 320, 512, 1024
PRED_P = 384  # PRED_H zero-padded to a multiple of 128
G = U * T  # 20200 grid points per core, u-major: g = u*T + t
GW = U + JH  # g and w1g packed side by side, one DMA
LS = 16.0  # lambda_s: h-side scale, folded into W1/b1 on host
LW = 16.0  # lambda_w: W2 scale for the e4m3 copy
# PSUM = LS*LW*(s2@W2) = 256*(s2@W2); |256*s2@W2| < ~120 so int8 = RNE(psum)
LOUT = LS * LW
INV_PHI = 0.3989422804014327  # 1/sqrt(2*pi)
SC_DRAIN_VCS = (0, 1, 2, 3, 4, 5, 6, 7)  # vocab chunks drained on ScalarE

F32 = mybir.dt.float32
F16 = mybir.dt.float16
F8 = mybir.dt.float8e4
I8 = mybir.dt.int8
AF = mybir.ActivationFunctionType
ALU = mybir.AluOpType
DR = mybir.MatmulPerfMode.DoubleRow

_CACHE = {}


def _build_program():
    nc = bacc.Bacc(None, target_bir_lowering=False)

    fP = nc.declare_dram_parameter("fP", [128, 8, T], F16, isOutput=False)
    gwP = nc.declare_dram_parameter("gwP", [128, 3, GW], F16, isOutput=False)
    w1fP = nc.declare_dram_parameter("w1fP", [128, 8, JH], F16, isOutput=False)
    w2P = nc.declare_dram_parameter("w2P", [128, 4, V], F16, isOutput=False)
    w8P = nc.declare_dram_parameter("w8P", [128, 4, V], F8, isOutput=False)
    bc = nc.declare_dram_parameter("bc", [128, 4], F32, isOutput=False)
    outT = nc.declare_dram_parameter("outT", [V, G], I8, isOutput=True)
    mtwP = nc.declare_dram_parameter("mtwP", [128, 8, T], F32, isOutput=True)
    cuwP = nc.declare_dram_parameter("cuwP", [128, 8, U], F32, isOutput=True)

    with tile.TileContext(nc) as tc:
        with (
            tc.tile_pool(name="const", bufs=1) as const,
            tc.tile_pool(name="hbuf", bufs=3) as hbuf,
            tc.tile_pool(name="obuf", bufs=6) as obuf,
            tc.tile_pool(name="psum", bufs=2, space="PSUM") as psum,
        ):
            # ---- load inputs (small tensors first; HWDGE ring drains FIFO) ----
            gw_sb = const.tile([128, 3, GW], F16, tag="gw_sb")
            for c in range(3):
                nc.sync.dma_start(gw_sb[:, c, :], gwP[:, c, :])
            b_sb = const.tile([128, 4], F32, tag="b_sb")
            nc.sync.dma_start(b_sb[:, :], bc[:, :])
            f_sb = const.tile([128, 8, T], F16, tag="f_sb")
            w1f_sb = const.tile([128, 8, JH], F16, tag="w1f_sb")
            w8_sb = const.tile([128, 4, V], F8, tag="w8_sb")
            w2_sb = const.tile([128, 4, V], F16, tag="w2_sb")
            for q in range(4):
                nc.sync.dma_start(
                    f_sb[:, 2 * q : 2 * q + 2, :], fP[:, 2 * q : 2 * q + 2, :]
                )
                nc.sync.dma_start(
                    w1f_sb[:, 2 * q : 2 * q + 2, :], w1fP[:, 2 * q : 2 * q + 2, :]
                )
                nc.sync.dma_start(w8_sb[:, q, :], w8P[:, q, :])
            for q in range(4):
                nc.sync.dma_start(w2_sb[:, q, :], w2P[:, q, :])

            # ---- first-layer projections (LS-scaled via host-side W1/b1) ----
            pg_ps = psum.tile([128, 4, 512], F32, tag="pt")
            for jc in range(4):
                for c in range(3):
                    nc.tensor.matmul(
                        pg_ps[:, jc, :U],
                        gw_sb[:, c, U + jc * 128 : U + (jc + 1) * 128],
                        gw_sb[:, c, :U],
                        start=(c == 0),
                        stop=(c == 2),
                    )
            pg_sb = const.tile([128, 4, U], F32, tag="pg_sb")
            for jc in range(4):
                nc.vector.tensor_scalar(
                    pg_sb[:, jc, :],
                    pg_ps[:, jc, :U],
                    b_sb[:, jc : jc + 1],
                    None,
                    ALU.add,
                )
            pf_ps = psum.tile([128, 4, 512], F32, tag="pt")
            for hc in range(8):
                for jc in range(4):
                    nc.tensor.matmul(
                        pf_ps[:, jc, :T],
                        w1f_sb[:, hc, jc * 128 : (jc + 1) * 128],
                        f_sb[:, hc, :],
                        start=(hc == 0),
                        stop=(hc == 7),
                    )
            pf16 = const.tile([128, 4, T], F16, tag="pf16")
            for jc in range(4):
                nc.vector.tensor_copy(pf16[:, jc, :], pf_ps[:, jc, :T])

            # ---- per-j moments over t (of pf) and u (of pg) ----
            stat = const.tile([128, 22, 4], F32, tag="stat")
            (
                mu_f, e2_f, msq_f, var_f, sd_f, rs_f, mu_g, e2_g, msq_g, var_g,
                sd_g, rs_g, g0, vs_, sg_, rg_, ag, gg, sqg, eg, tg, grand,
            ) = (stat[:, i, :] for i in range(22))

            pfsq = const.tile([128, 4, T], F32, tag="pfsq")
            nc.vector.tensor_tensor(pfsq[:, :, :], pf16[:, :, :], pf16[:, :, :], ALU.mult)
            nc.vector.tensor_reduce(mu_f[:, :], pf16[:, :, :], mybir.AxisListType.X, ALU.add)
            nc.vector.tensor_reduce(e2_f[:, :], pfsq[:, :, :], mybir.AxisListType.X, ALU.add)
            pgsq = const.tile([128, 4, U], F32, tag="pgsq")
            nc.vector.tensor_tensor(pgsq[:, :, :], pg_sb[:, :, :], pg_sb[:, :, :], ALU.mult)
            nc.vector.tensor_reduce(mu_g[:, :], pg_sb[:, :, :], mybir.AxisListType.X, ALU.add)
            nc.vector.tensor_reduce(e2_g[:, :], pgsq[:, :, :], mybir.AxisListType.X, ALU.add)
            nc.vector.tensor_scalar(mu_f[:, :], mu_f[:, :], 1.0 / T, None, ALU.mult)
            nc.vector.tensor_scalar(e2_f[:, :], e2_f[:, :], 1.0 / T, None, ALU.mult)
            nc.vector.tensor_scalar(mu_g[:, :], mu_g[:, :], 1.0 / U, None, ALU.mult)
            nc.vector.tensor_scalar(e2_g[:, :], e2_g[:, :], 1.0 / U, None, ALU.mult)
            nc.vector.tensor_tensor(msq_f[:, :], mu_f[:, :], mu_f[:, :], ALU.mult)
            nc.vector.tensor_tensor(msq_g[:, :], mu_g[:, :], mu_g[:, :], ALU.mult)
            nc.vector.scalar_tensor_tensor(
                var_f[:, :], e2_f[:, :], 1e-5, msq_f[:, :], ALU.add, ALU.subtract
            )
            nc.vector.scalar_tensor_tensor(
                var_g[:, :], e2_g[:, :], 1e-5, msq_g[:, :], ALU.add, ALU.subtract
            )
            nc.scalar.activation(sd_f[:, :], var_f[:, :], AF.Sqrt)
            nc.vector.reciprocal(rs_f[:, :], sd_f[:, :])
            nc.scalar.activation(sd_g[:, :], var_g[:, :], AF.Sqrt)
            nc.vector.reciprocal(rs_g[:, :], sd_g[:, :])
            nc.vector.tensor_tensor(g0[:, :], mu_f[:, :], mu_g[:, :], ALU.add)
            nc.vector.tensor_tensor(vs_[:, :], var_f[:, :], var_g[:, :], ALU.add)
            nc.scalar.activation(sg_[:, :], vs_[:, :], AF.Sqrt)
            nc.vector.reciprocal(rg_[:, :], sg_[:, :])

            # ---- analytic mean fields: psi(zh) = Gelu(zh) + phi(zh) ----
            # (Gelu/Exp calls batched so ScalarE loads each ACT table once.)
            # mt[j,t] = sd_g * psi((pf + mu_g)/sd_g)   (E_u[relu(x)])
            zh = const.tile([128, 4, T], F32, tag="zh")
            for jc in range(4):
                nc.vector.tensor_scalar(
                    zh[:, jc, :],
                    pf16[:, jc, :],
                    mu_g[:, jc : jc + 1],
                    rs_g[:, jc : jc + 1],
                    ALU.add,
                    ALU.mult,
                )
            # yh for cu[j,u] = sd_f * psi((pg + mu_f)/sd_f) - grand
            yh = const.tile([128, 4, U], F32, tag="yh")
            for jc in range(4):
                nc.vector.tensor_scalar(
                    yh[:, jc, :],
                    pg_sb[:, jc, :],
                    mu_f[:, jc : jc + 1],
                    rs_f[:, jc : jc + 1],
                    ALU.add,
                    ALU.mult,
                )
            nc.vector.tensor_tensor(ag[:, :], g0[:, :], rg_[:, :], ALU.mult)
            g1t = const.tile([128, 4, T], F32, tag="g1t")
            g1u = const.tile([128, 4, U], F32, tag="g1u")
            nc.scalar.activation(g1t[:, :, :], zh[:, :, :], AF.Gelu)
            nc.scalar.activation(g1u[:, :, :], yh[:, :, :], AF.Gelu)
            nc.scalar.activation(gg[:, :], ag[:, :], AF.Gelu)
            sqt = pfsq  # reuse
            nc.vector.tensor_tensor(sqt[:, :, :], zh[:, :, :], zh[:, :, :], ALU.mult)
            squ = pgsq  # reuse
            nc.vector.tensor_tensor(squ[:, :, :], yh[:, :, :], yh[:, :, :], ALU.mult)
            nc.vector.tensor_tensor(sqg[:, :], ag[:, :], ag[:, :], ALU.mult)
            e1t = const.tile([128, 4, T], F32, tag="e1t")
            e1u = const.tile([128, 4, U], F32, tag="e1u")
            nc.scalar.activation(e1t[:, :, :], sqt[:, :, :], AF.Exp, scale=-0.5)
            nc.scalar.activation(e1u[:, :, :], squ[:, :, :], AF.Exp, scale=-0.5)
            nc.scalar.activation(eg[:, :], sqg[:, :], AF.Exp, scale=-0.5)
            tmpt = zh  # reuse
            nc.vector.scalar_tensor_tensor(
                tmpt[:, :, :], e1t[:, :, :], INV_PHI, g1t[:, :, :], ALU.mult, ALU.add
            )
            mt16 = const.tile([128, 4, T], F16, tag="mt16")
            for jc in range(4):
                nc.vector.tensor_scalar(
                    mt16[:, jc, :], tmpt[:, jc, :], sd_g[:, jc : jc + 1], None, ALU.mult
                )
            tmpu = yh  # reuse
            nc.vector.scalar_tensor_tensor(
                tmpu[:, :, :], e1u[:, :, :], INV_PHI, g1u[:, :, :], ALU.mult, ALU.add
            )
            nc.vector.scalar_tensor_tensor(
                tg[:, :], eg[:, :], INV_PHI, gg[:, :], ALU.mult, ALU.add
            )
            nc.vector.tensor_tensor(grand[:, :], tg[:, :], sg_[:, :], ALU.mult)

            cu16 = const.tile([128, 4, U], F16, tag="cu16")
            for jc in range(4):
                nc.vector.tensor_scalar(
                    cu16[:, jc, :],
                    tmpu[:, jc, :],
                    sd_f[:, jc : jc + 1],
                    grand[:, jc : jc + 1],
                    ALU.mult,
                    ALU.subtract,
                )
            cun32 = const.tile([128, 4, U], F32, tag="cun32")
            nc.vector.tensor_scalar(cun32[:, :, :], cu16[:, :, :], -1.0, None, ALU.mult)
            pgc = const.tile([128, 4, U], F32, tag="pgc")
            nc.vector.tensor_tensor(pgc[:, :, :], pg_sb[:, :, :], cu16[:, :, :], ALU.subtract)

            # ---- rank images mt@W2, cu@W2 (fp16 matmuls; host adds them) ----
            rk1 = psum.tile([128, 4, 512], F32, tag="pt")
            for vc in range(8):
                sl = rk1[:, vc // 2, (vc % 2) * 256 : (vc % 2) * 256 + T]
                for jc in range(4):
                    nc.tensor.matmul(
                        sl,
                        w2_sb[:, jc, vc * 128 : (vc + 1) * 128],
                        mt16[:, jc, :],
                        start=(jc == 0),
                        stop=(jc == 3),
                    )
            mtw_sb = const.tile([128, 8, T], F32, tag="mtw_sb")
            for vc in range(8):
                nc.scalar.activation(
                    mtw_sb[:, vc, :],
                    rk1[:, vc // 2, (vc % 2) * 256 : (vc % 2) * 256 + T],
                    AF.Identity,
                )
            nc.sync.dma_start(mtwP[:, :, :], mtw_sb[:, :, :])
            rk2 = psum.tile([128, 4, 512], F32, tag="pt")
            for vc in range(8):
                sl = rk2[:, vc // 2, (vc % 2) * 256 : (vc % 2) * 256 + U]
                for jc in range(4):
                    nc.tensor.matmul(
                        sl,
                        w2_sb[:, jc, vc * 128 : (vc + 1) * 128],
                        cu16[:, jc, :],
                        start=(jc == 0),
                        stop=(jc == 3),
                    )
            cuw_sb = const.tile([128, 8, U], F32, tag="cuw_sb")
            for vc in range(8):
                nc.scalar.activation(
                    cuw_sb[:, vc, :],
                    rk2[:, vc // 2, (vc % 2) * 256 : (vc % 2) * 256 + U],
                    AF.Identity,
                )
            nc.sync.dma_start(cuwP[:, :, :], cuw_sb[:, :, :])

            # ---- main loop: spans of 10 u-rows (2000 cols); tail (u=100) first
            spans = [(100, 1)] + [(10 * s, 10) for s in range(10)]

            def emit_hgen(si):
                """s-gen for span si; returns the two fp8 h tile-pairs."""
                u0, nu = spans[si]
                glen = nu * T
                h0 = hbuf.tile([128, 2, 2000], F8, tag="h0")
                h1 = hbuf.tile([128, 2, 2000], F8, tag="h1")
                for jc in range(4):
                    hp = h0 if jc < 2 else h1
                    half = jc % 2
                    asp = hbuf.tile([128, 10, T], F16, tag=f"a{jc}")
                    for ui in range(nu):
                        u = u0 + ui
                        # a = max(pf + (pg-cu), -cu) = relu(pf+pg) - cu
                        nc.vector.tensor_scalar(
                            asp[:, ui, :],
                            pf16[:, jc, :],
                            pgc[:, jc, u : u + 1],
                            cun32[:, jc, u : u + 1],
                            ALU.add,
                            ALU.max,
                        )
                    # s8 = e4m3(a - mt): one span-wide op, mt broadcast over u
                    nc.vector.tensor_tensor(
                        hp[:, half, :glen].rearrange("p (a b) -> p a b", b=T),
                        asp[:, :nu, :],
                        mt16[:, jc : jc + 1, :].broadcast_to([128, nu, T]),
                        ALU.subtract,
                    )
                return h0, h1

            # software pipeline: h-gen for span si+1 is emitted before the
            # matmuls/drains of span si, so the producer-side DVE/ScalarE ops
            # sit ahead of span si's drains in each engine's FIFO queue.
            cur = emit_hgen(0)
            for si, (u0, nu) in enumerate(spans):
                glen = nu * T
                last = si == len(spans) - 1
                tailish = si >= len(spans) - 2
                h0, h1 = cur
                if not last:
                    cur = emit_hgen(si + 1)
                nb = (glen + 499) // 500
                for vc in range(8):
                    pt = psum.tile([128, 4, 512], F32, tag="pt")
                    for p in (0, 1):
                        hp = h0 if p == 0 else h1
                        for b in range(nb):
                            blen = min(500, glen - b * 500)
                            nc.tensor.matmul(
                                pt[:, b, :blen],
                                w8_sb[:, 2 * p : 2 * p + 2, vc * 128 : (vc + 1) * 128],
                                hp[:, :, b * 500 : b * 500 + blen],
                                start=(p == 0),
                                stop=(p == 1),
                                perf_mode=DR,
                            )
                    ob = obuf.tile([128, 4, 500], I8, tag="ob")
                    g0c = u0 * T
                    if last and vc == 7:
                        # final drain: split so the first half's DMA overlaps
                        # the second half's drain
                        for hb in (0, 2):
                            nc.scalar.activation(
                                ob[:, hb : hb + 2, :], pt[:, hb : hb + 2, :500], AF.Identity
                            )
                            nc.scalar.dma_start(
                                outT[
                                    vc * 128 : (vc + 1) * 128,
                                    g0c + hb * 500 : g0c + (hb + 2) * 500,
                                ],
                                ob[:, hb : hb + 2, :],
                            )
                        continue
                    if nu == 1:
                        src, dst = pt[:, 0, :glen], ob[:, 0, :glen]
                    else:
                        src, dst = pt[:, :, :500], ob[:, :, :]
                    if (vc in SC_DRAIN_VCS) and not (tailish and vc % 2 == 1):
                        nc.scalar.activation(dst, src, AF.Identity)
                    else:
                        nc.vector.tensor_copy(dst, src)
                    deng = nc.sync if vc % 2 == 0 else nc.scalar
                    deng.dma_start(
                        outT[vc * 128 : (vc + 1) * 128, g0c : g0c + glen], dst
                    )

    nc.compile()
    return nc


def _get_program():
    if "nc" not in _CACHE:
        _CACHE["nc"] = _build_program()
    return _CACHE["nc"]


def _pmajor(mT, nchunks):
    """[nchunks*128, free] -> [128, nchunks, free] partition-major layout."""
    free = mT.shape[1]
    return np.ascontiguousarray(mT.reshape(nchunks, 128, free).transpose(1, 0, 2))


def _prep_inputs(f, g, W1, b1, W2, b2):
    f16 = np.float16
    W1fT = (W1[:, :ENC_H].T * LS).astype(f16)  # (1024, 512), LS-scaled
    W1gT = np.zeros((PRED_P, JH), dtype=f16)
    W1gT[:PRED_H] = (W1[:, ENC_H:].T * LS).astype(f16)
    W2T = W2.T.astype(f16)  # (512, 1024) unscaled, for the rank matmuls
    w1fP = _pmajor(W1fT, 8)
    w1gP = _pmajor(W1gT, 3)
    w2P = _pmajor(W2T, 4)
    w8P = _pmajor((W2.T * LW).astype(np.float32).astype(ml_dtypes.float8_e4m3), 4)
    bc = np.ascontiguousarray((LS * b1).reshape(4, 128).T.astype(np.float32))
    in_maps = []
    for i in range(B):
        gTp = np.zeros((PRED_P, U), dtype=f16)
        gTp[:PRED_H] = g[i].T.astype(f16)
        gwP = np.empty((128, 3, GW), dtype=f16)
        gwP[:, :, :U] = _pmajor(gTp, 3)
        gwP[:, :, U:] = w1gP
        in_maps.append(
            {
                "fP": _pmajor(f[i].T.astype(f16), 8),
                "gwP": gwP,
                "w1fP": w1fP,
                "w2P": w2P,
                "w8P": w8P,
                "bc": bc,
            }
        )
    return in_maps


def run_on_device(f, g, W1, b1, W2, b2, **spmd_kwargs):
    """Runs the kernel; returns (logits, BassKernelResults)."""
    nc = _get_program()
    in_maps = _prep_inputs(f, g, W1, b1, W2, b2)
    res = run_bass_kernel_spmd(nc, in_maps, list(range(B)), **spmd_kwargs)
    out = np.empty((B, T, U, V), dtype=np.float32)
    inv = np.float32(1.0 / LOUT)
    inv_ls = np.float32(1.0 / LS)
    b2f = b2.astype(np.float32)
    for i in range(B):
        r = res.results[i]
        full = r["outT"].astype(np.float32).reshape(V, U, T) * inv  # s2@W2 part
        mtw = r["mtwP"].transpose(1, 0, 2).reshape(V, T) * inv_ls  # (V,T)
        cuw = r["cuwP"].transpose(1, 0, 2).reshape(V, U) * inv_ls  # (V,U)
        full += mtw[:, None, :]
        full += cuw[:, :, None]
        full += b2f[:, None, None]
        out[i] = full.transpose(2, 1, 0)
    return out, res


def kernel(f, g, W1, b1, W2, b2):
    out, _ = run_on_device(f, g, W1, b1, W2, b2)
    return out


# revision 20
# speedup vs baseline: 1.0252x; 1.0252x over previous
"""RNN-T joint network kernel for Trainium2 (8 NeuronCores, data-parallel over B).

Computes logits = relu(f @ W1f.T + g @ W1g.T + b1) @ W2.T + b2 over the
(B, T, U, ...) broadcast grid without materializing the concat tensor.

v2 strategy (per core, one batch element b) — fp8 DoubleRow main matmul:
  - The dominant cost is h @ W2.T (10.6 GMAC/core).  fp16 runs it at
    ~1 col/cycle/128-chunk on the PE; fp8e4 (e4m3) with
    MatmulPerfMode.DoubleRow contracts 256 K per column-cycle => 2x.
  - Direct e4m3 quantization of h fails the 2e-2 gate (0.044).  Fix:
    subtract a rank-structured mean field before quantizing.  With
    x = pf[t,j] + pg[u,j] (+b1) and q = relu(x):
        s2[j,t,u] = q - mt[j,t] - cu[j,u]
    where mt[j,t] ~ E_u[q] and cu[j,u] ~ E_t[q] - E[q] are the two-way
    ANOVA main effects, computed in closed form on-device:
    pf[.,j]/pg[.,j] are (exactly) Gaussian across t/u, so
    E[relu(z + sigma Z)] = sigma*(phi(zh) + zh*Phi(zh)) = sigma*(0.3989*
    exp(-zh^2/2) + Gelu(zh)) — ScalarE Gelu/Exp + tiny DVE ops.
    s2 has ~3x smaller magnitude than q, so e4m3(s2) passes easily
    (measured end-to-end rel err ~0.012 vs gate 2e-2).
  - s-gen is fully fused per (jc, u):  a = max(pf + (pg-cu), -cu)
    (= relu(x) - cu, one DVE tensor_scalar with two per-partition AP
    scalars), then s8 = e4m3(a - mt) (one tensor_tensor, fp8 out).
  - Scales: lambda_s=16 folded into W1/b1 on host, lambda_w=16 into the
    e4m3 W2 copy => PSUM = 256 * (s2@W2) and the int8 output scale is
    exactly 256: the drain is a bare fp32->int8 cast (ScalarE Identity
    activation for 7 vocab chunks, DVE tensor_copy for 1).
  - The rank images mt@W2 (V x T) and cu@W2 (V x U) are computed on
    device with tiny fp16 matmuls and DMA'd out; the host adds them
    (plus b2) during the existing dequant/transpose pass.
  - Grid u-major: g = u*T + t; spans of 10 u-rows (2000 cols, 4 PSUM
    banks of 500); the single-u tail span (u=100) is processed first to
    prime the pipeline.
"""

import sys

sys.path.insert(0, "/opt/trn_rl_repo")

import ml_dtypes
import numpy as np

from concourse import bacc, bass, tile, mybir
from concourse.bass_utils import run_bass_kernel_spmd

B, T, U = 8, 200, 101
ENC_H, PRED_H, JH, V = 1024, 320, 512, 1024
PRED_P = 384  # PRED_H zero-padded to a multiple of 128
G = U * T  # 20200 grid points per core, u-major: g = u*T + t
GW = U + JH  # g and w1g packed side by side, one DMA
LS = 16.0  # lambda_s: h-side scale, folded into W1/b1 on host
LW = 16.0  # lambda_w: W2 scale for the e4m3 copy
# PSUM = LS*LW*(s2@W2) = 256*(s2@W2); |256*s2@W2| < ~120 so int8 = RNE(psum)
LOUT = LS * LW
INV_PHI = 0.3989422804014327  # 1/sqrt(2*pi)
SC_DRAIN_VCS = (0, 1, 2, 3, 4, 5, 6, 7)  # vocab chunks drained on ScalarE

F32 = mybir.dt.float32
F16 = mybir.dt.float16
F8 = mybir.dt.float8e4
I8 = mybir.dt.int8
AF = mybir.ActivationFunctionType
ALU = mybir.AluOpType
DR = mybir.MatmulPerfMode.DoubleRow

_CACHE = {}


def _build_program():
    nc = bacc.Bacc(None, target_bir_lowering=False)

    fP = nc.declare_dram_parameter("fP", [128, 8, T], F16, isOutput=False)
    gwP = nc.declare_dram_parameter("gwP", [128, 3, GW], F16, isOutput=False)
    w1fP = nc.declare_dram_parameter("w1fP", [128, 8, JH], F16, isOutput=False)
    w2P = nc.declare_dram_parameter("w2P", [128, 4, V], F16, isOutput=False)
    w8P = nc.declare_dram_parameter("w8P", [128, 4, V], F8, isOutput=False)
    bc = nc.declare_dram_parameter("bc", [128, 4], F32, isOutput=False)
    outT = nc.declare_dram_parameter("outT", [V, G], I8, isOutput=True)
    mtwP = nc.declare_dram_parameter("mtwP", [128, 8, T], F32, isOutput=True)
    cuwP = nc.declare_dram_parameter("cuwP", [128, 8, U], F32, isOutput=True)

    with tile.TileContext(nc) as tc:
        with (
            tc.tile_pool(name="const", bufs=1) as const,
            tc.tile_pool(name="hbuf", bufs=3) as hbuf,
            tc.tile_pool(name="obuf", bufs=6) as obuf,
            tc.tile_pool(name="psum", bufs=2, space="PSUM") as psum,
        ):
            # ---- warm the Sqrt ACT table while input DMAs run ----
            warm = const.tile([128, 2], F32, tag="warm")
            nc.vector.memset(warm[:, :], 1.0)
            nc.scalar.activation(warm[:, 1:2], warm[:, 0:1], AF.Sqrt)

            # ---- load inputs (small tensors first; HWDGE ring drains FIFO) ----
            gw_sb = const.tile([128, 3, GW], F16, tag="gw_sb")
            for c in range(3):
                nc.sync.dma_start(gw_sb[:, c, :], gwP[:, c, :])
            b_sb = const.tile([128, 4], F32, tag="b_sb")
            nc.sync.dma_start(b_sb[:, :], bc[:, :])
            f_sb = const.tile([128, 8, T], F16, tag="f_sb")
            w1f_sb = const.tile([128, 8, JH], F16, tag="w1f_sb")
            w8_sb = const.tile([128, 4, V], F8, tag="w8_sb")
            w2_sb = const.tile([128, 4, V], F16, tag="w2_sb")
            for q in range(4):
                nc.sync.dma_start(
                    f_sb[:, 2 * q : 2 * q + 2, :], fP[:, 2 * q : 2 * q + 2, :]
                )
                nc.sync.dma_start(
                    w1f_sb[:, 2 * q : 2 * q + 2, :], w1fP[:, 2 * q : 2 * q + 2, :]
                )
                nc.sync.dma_start(w8_sb[:, q, :], w8P[:, q, :])
            for q in range(4):
                nc.sync.dma_start(w2_sb[:, q, :], w2P[:, q, :])

            # ---- first-layer projections (LS-scaled via host-side W1/b1) ----
            pg_ps = psum.tile([128, 4, 512], F32, tag="pt")
            for jc in range(4):
                for c in range(3):
                    nc.tensor.matmul(
                        pg_ps[:, jc, :U],
                        gw_sb[:, c, U + jc * 128 : U + (jc + 1) * 128],
                        gw_sb[:, c, :U],
                        start=(c == 0),
                        stop=(c == 2),
                    )
            pg_sb = const.tile([128, 4, U], F32, tag="pg_sb")
            for jc in range(4):
                nc.vector.tensor_scalar(
                    pg_sb[:, jc, :],
                    pg_ps[:, jc, :U],
                    b_sb[:, jc : jc + 1],
                    None,
                    ALU.add,
                )
            pf_ps = psum.tile([128, 4, 512], F32, tag="pt")
            for hc in range(8):
                for jc in range(4):
                    nc.tensor.matmul(
                        pf_ps[:, jc, :T],
                        w1f_sb[:, hc, jc * 128 : (jc + 1) * 128],
                        f_sb[:, hc, :],
                        start=(hc == 0),
                        stop=(hc == 7),
                    )
            pf16 = const.tile([128, 4, T], F16, tag="pf16")
            for jc in range(4):
                nc.vector.tensor_copy(pf16[:, jc, :], pf_ps[:, jc, :T])

            # ---- per-j moments over t (of pf) and u (of pg) ----
            stat = const.tile([128, 22, 4], F32, tag="stat")
            (
                mu_f, e2_f, msq_f, var_f, sd_f, rs_f, mu_g, e2_g, msq_g, var_g,
                sd_g, rs_g, g0, vs_, sg_, rg_, ag, gg, sqg, eg, tg, grand,
            ) = (stat[:, i, :] for i in range(22))

            pfsq = const.tile([128, 4, T], F32, tag="pfsq")
            nc.vector.tensor_tensor(pfsq[:, :, :], pf16[:, :, :], pf16[:, :, :], ALU.mult)
            nc.vector.tensor_reduce(mu_f[:, :], pf16[:, :, :], mybir.AxisListType.X, ALU.add)
            nc.vector.tensor_reduce(e2_f[:, :], pfsq[:, :, :], mybir.AxisListType.X, ALU.add)
            pgsq = const.tile([128, 4, U], F32, tag="pgsq")
            nc.vector.tensor_tensor(pgsq[:, :, :], pg_sb[:, :, :], pg_sb[:, :, :], ALU.mult)
            nc.vector.tensor_reduce(mu_g[:, :], pg_sb[:, :, :], mybir.AxisListType.X, ALU.add)
            nc.vector.tensor_reduce(e2_g[:, :], pgsq[:, :, :], mybir.AxisListType.X, ALU.add)
            nc.vector.tensor_scalar(mu_f[:, :], mu_f[:, :], 1.0 / T, None, ALU.mult)
            nc.vector.tensor_scalar(e2_f[:, :], e2_f[:, :], 1.0 / T, None, ALU.mult)
            nc.vector.tensor_scalar(mu_g[:, :], mu_g[:, :], 1.0 / U, None, ALU.mult)
            nc.vector.tensor_scalar(e2_g[:, :], e2_g[:, :], 1.0 / U, None, ALU.mult)
            nc.vector.tensor_tensor(msq_f[:, :], mu_f[:, :], mu_f[:, :], ALU.mult)
            nc.vector.tensor_tensor(msq_g[:, :], mu_g[:, :], mu_g[:, :], ALU.mult)
            nc.vector.scalar_tensor_tensor(
                var_f[:, :], e2_f[:, :], 1e-5, msq_f[:, :], ALU.add, ALU.subtract
            )
            nc.vector.scalar_tensor_tensor(
                var_g[:, :], e2_g[:, :], 1e-5, msq_g[:, :], ALU.add, ALU.subtract
            )
            nc.scalar.activation(sd_f[:, :], var_f[:, :], AF.Sqrt)
            nc.vector.reciprocal(rs_f[:, :], sd_f[:, :])
            nc.scalar.activation(sd_g[:, :], var_g[:, :], AF.Sqrt)
            nc.vector.reciprocal(rs_g[:, :], sd_g[:, :])
            nc.vector.tensor_tensor(g0[:, :], mu_f[:, :], mu_g[:, :], ALU.add)
            nc.vector.tensor_tensor(vs_[:, :], var_f[:, :], var_g[:, :], ALU.add)
            nc.scalar.activation(sg_[:, :], vs_[:, :], AF.Sqrt)
            nc.vector.reciprocal(rg_[:, :], sg_[:, :])

            # ---- analytic mean fields: psi(zh) = Gelu(zh) + phi(zh) ----
            # (Gelu/Exp calls batched so ScalarE loads each ACT table once.)
            # mt[j,t] = sd_g * psi((pf + mu_g)/sd_g)   (E_u[relu(x)])
            zh = const.tile([128, 4, T], F32, tag="zh")
            for jc in range(4):
                nc.vector.tensor_scalar(
                    zh[:, jc, :],
                    pf16[:, jc, :],
                    mu_g[:, jc : jc + 1],
                    rs_g[:, jc : jc + 1],
                    ALU.add,
                    ALU.mult,
                )
            # yh for cu[j,u] = sd_f * psi((pg + mu_f)/sd_f) - grand
            yh = const.tile([128, 4, U], F32, tag="yh")
            for jc in range(4):
                nc.vector.tensor_scalar(
                    yh[:, jc, :],
                    pg_sb[:, jc, :],
                    mu_f[:, jc : jc + 1],
                    rs_f[:, jc : jc + 1],
                    ALU.add,
                    ALU.mult,
                )
            nc.vector.tensor_tensor(ag[:, :], g0[:, :], rg_[:, :], ALU.mult)
            g1t = const.tile([128, 4, T], F32, tag="g1t")
            g1u = const.tile([128, 4, U], F32, tag="g1u")
            nc.scalar.activation(g1t[:, :, :], zh[:, :, :], AF.Gelu)
            nc.scalar.activation(g1u[:, :, :], yh[:, :, :], AF.Gelu)
            nc.scalar.activation(gg[:, :], ag[:, :], AF.Gelu)
            sqt = pfsq  # reuse
            nc.vector.tensor_tensor(sqt[:, :, :], zh[:, :, :], zh[:, :, :], ALU.mult)
            squ = pgsq  # reuse
            nc.vector.tensor_tensor(squ[:, :, :], yh[:, :, :], yh[:, :, :], ALU.mult)
            nc.vector.tensor_tensor(sqg[:, :], ag[:, :], ag[:, :], ALU.mult)
            e1t = const.tile([128, 4, T], F32, tag="e1t")
            e1u = const.tile([128, 4, U], F32, tag="e1u")
            nc.scalar.activation(e1t[:, :, :], sqt[:, :, :], AF.Exp, scale=-0.5)
            nc.scalar.activation(e1u[:, :, :], squ[:, :, :], AF.Exp, scale=-0.5)
            nc.scalar.activation(eg[:, :], sqg[:, :], AF.Exp, scale=-0.5)
            tmpt = zh  # reuse
            nc.vector.scalar_tensor_tensor(
                tmpt[:, :, :], e1t[:, :, :], INV_PHI, g1t[:, :, :], ALU.mult, ALU.add
            )
            mt16 = const.tile([128, 4, T], F16, tag="mt16")
            for jc in range(4):
                nc.vector.tensor_scalar(
                    mt16[:, jc, :], tmpt[:, jc, :], sd_g[:, jc : jc + 1], None, ALU.mult
                )
            tmpu = yh  # reuse
            nc.vector.scalar_tensor_tensor(
                tmpu[:, :, :], e1u[:, :, :], INV_PHI, g1u[:, :, :], ALU.mult, ALU.add
            )
            nc.vector.scalar_tensor_tensor(
                tg[:, :], eg[:, :], INV_PHI, gg[:, :], ALU.mult, ALU.add
            )
            nc.vector.tensor_tensor(grand[:, :], tg[:, :], sg_[:, :], ALU.mult)

            cu16 = const.tile([128, 4, U], F16, tag="cu16")
            for jc in range(4):
                nc.vector.tensor_scalar(
                    cu16[:, jc, :],
                    tmpu[:, jc, :],
                    sd_f[:, jc : jc + 1],
                    grand[:, jc : jc + 1],
                    ALU.mult,
                    ALU.subtract,
                )
            cun32 = const.tile([128, 4, U], F32, tag="cun32")
            nc.vector.tensor_scalar(cun32[:, :, :], cu16[:, :, :], -1.0, None, ALU.mult)
            pgc = const.tile([128, 4, U], F32, tag="pgc")
            nc.vector.tensor_tensor(pgc[:, :, :], pg_sb[:, :, :], cu16[:, :, :], ALU.subtract)

            # ---- rank images mt@W2, cu@W2 (fp16 matmuls; host adds them) ----
            rk1 = psum.tile([128, 4, 512], F32, tag="pt")
            for vc in range(8):
                sl = rk1[:, vc // 2, (vc % 2) * 256 : (vc % 2) * 256 + T]
                for jc in range(4):
                    nc.tensor.matmul(
                        sl,
                        w2_sb[:, jc, vc * 128 : (vc + 1) * 128],
                        mt16[:, jc, :],
                        start=(jc == 0),
                        stop=(jc == 3),
                    )
            mtw_sb = const.tile([128, 8, T], F32, tag="mtw_sb")
            for vc in range(8):
                nc.scalar.activation(
                    mtw_sb[:, vc, :],
                    rk1[:, vc // 2, (vc % 2) * 256 : (vc % 2) * 256 + T],
                    AF.Identity,
                )
            nc.sync.dma_start(mtwP[:, :, :], mtw_sb[:, :, :])
            rk2 = psum.tile([128, 4, 512], F32, tag="pt")
            for vc in range(8):
                sl = rk2[:, vc // 2, (vc % 2) * 256 : (vc % 2) * 256 + U]
                for jc in range(4):
                    nc.tensor.matmul(
                        sl,
                        w2_sb[:, jc, vc * 128 : (vc + 1) * 128],
                        cu16[:, jc, :],
                        start=(jc == 0),
                        stop=(jc == 3),
                    )
            cuw_sb = const.tile([128, 8, U], F32, tag="cuw_sb")
            for vc in range(8):
                nc.scalar.activation(
                    cuw_sb[:, vc, :],
                    rk2[:, vc // 2, (vc % 2) * 256 : (vc % 2) * 256 + U],
                    AF.Identity,
                )
            nc.sync.dma_start(cuwP[:, :, :], cuw_sb[:, :, :])

            # ---- main loop: spans of 10 u-rows (2000 cols); tail (u=100) first
            spans = (
                [(100, 1)] + [(10 * s, 10) for s in range(9)] + [(90, 5), (95, 5)]
            )

            def emit_hgen(si):
                """s-gen for span si; returns the two fp8 h tile-pairs."""
                u0, nu = spans[si]
                glen = nu * T
                h0 = hbuf.tile([128, 2, 2000], F8, tag="h0")
                h1 = hbuf.tile([128, 2, 2000], F8, tag="h1")
                for jc in range(4):
                    hp = h0 if jc < 2 else h1
                    half = jc % 2
                    asp = hbuf.tile([128, 10, T], F16, tag=f"a{jc}")
                    for ui in range(nu):
                        u = u0 + ui
                        # a = max(pf + (pg-cu), -cu) = relu(pf+pg) - cu
                        nc.vector.tensor_scalar(
                            asp[:, ui, :],
                            pf16[:, jc, :],
                            pgc[:, jc, u : u + 1],
                            cun32[:, jc, u : u + 1],
                            ALU.add,
                            ALU.max,
                        )
                    # s8 = e4m3(a - mt): one span-wide op, mt broadcast over u
                    nc.vector.tensor_tensor(
                        hp[:, half, :glen].rearrange("p (a b) -> p a b", b=T),
                        asp[:, :nu, :],
                        mt16[:, jc : jc + 1, :].broadcast_to([128, nu, T]),
                        ALU.subtract,
                    )
                return h0, h1

            # software pipeline: h-gen for span si+1 is emitted before the
            # matmuls/drains of span si, so the producer-side DVE/ScalarE ops
            # sit ahead of span si's drains in each engine's FIFO queue.
            cur = emit_hgen(0)
            for si, (u0, nu) in enumerate(spans):
                glen = nu * T
                last = si == len(spans) - 1
                h0, h1 = cur
                if not last:
                    cur = emit_hgen(si + 1)
                nb = (glen + 499) // 500
                for vc in range(8):
                    pt = psum.tile([128, 4, 512], F32, tag="pt")
                    for p in (0, 1):
                        hp = h0 if p == 0 else h1
                        for b in range(nb):
                            blen = min(500, glen - b * 500)
                            nc.tensor.matmul(
                                pt[:, b, :blen],
                                w8_sb[:, 2 * p : 2 * p + 2, vc * 128 : (vc + 1) * 128],
                                hp[:, :, b * 500 : b * 500 + blen],
                                start=(p == 0),
                                stop=(p == 1),
                                perf_mode=DR,
                            )
                    ob = obuf.tile([128, 4, 500], I8, tag="ob")
                    g0c = u0 * T
                    if last and vc == 7:
                        # final drain: split so the first half's DMA overlaps
                        # the second half's drain
                        nh = nb // 2
                        for hb in (0, nh):
                            hl = nh if hb == 0 else nb - nh
                            nc.scalar.activation(
                                ob[:, hb : hb + hl, :], pt[:, hb : hb + hl, :500], AF.Identity
                            )
                            nc.scalar.dma_start(
                                outT[
                                    vc * 128 : (vc + 1) * 128,
                                    g0c + hb * 500 : g0c + (hb + hl) * 500,
                                ],
                                ob[:, hb : hb + hl, :],
                            )
                        continue
                    if nu == 1:
                        src, dst = pt[:, 0, :glen], ob[:, 0, :glen]
                    else:
                        src, dst = pt[:, :nb, :500], ob[:, :nb, :]
                    if (vc in SC_DRAIN_VCS) and not (last and vc % 2 == 1):
                        nc.scalar.activation(dst, src, AF.Identity)
                    else:
                        nc.vector.tensor_copy(dst, src)
                    deng = nc.sync if vc % 2 == 0 else nc.scalar
                    deng.dma_start(
                        outT[vc * 128 : (vc + 1) * 128, g0c : g0c + glen], dst
                    )

    nc.compile()
    return nc


def _get_program():
    if "nc" not in _CACHE:
        _CACHE["nc"] = _build_program()
    return _CACHE["nc"]


def _pmajor(mT, nchunks):
    """[nchunks*128, free] -> [128, nchunks, free] partition-major layout."""
    free = mT.shape[1]
    return np.ascontiguousarray(mT.reshape(nchunks, 128, free).transpose(1, 0, 2))


def _prep_inputs(f, g, W1, b1, W2, b2):
    f16 = np.float16
    W1fT = (W1[:, :ENC_H].T * LS).astype(f16)  # (1024, 512), LS-scaled
    W1gT = np.zeros((PRED_P, JH), dtype=f16)
    W1gT[:PRED_H] = (W1[:, ENC_H:].T * LS).astype(f16)
    W2T = W2.T.astype(f16)  # (512, 1024) unscaled, for the rank matmuls
    w1fP = _pmajor(W1fT, 8)
    w1gP = _pmajor(W1gT, 3)
    w2P = _pmajor(W2T, 4)
    w8P = _pmajor((W2.T * LW).astype(np.float32).astype(ml_dtypes.float8_e4m3), 4)
    bc = np.ascontiguousarray((LS * b1).reshape(4, 128).T.astype(np.float32))
    in_maps = []
    for i in range(B):
        gTp = np.zeros((PRED_P, U), dtype=f16)
        gTp[:PRED_H] = g[i].T.astype(f16)
        gwP = np.empty((128, 3, GW), dtype=f16)
        gwP[:, :, :U] = _pmajor(gTp, 3)
        gwP[:, :, U:] = w1gP
        in_maps.append(
            {
                "fP": _pmajor(f[i].T.astype(f16), 8),
                "gwP": gwP,
                "w1fP": w1fP,
                "w2P": w2P,
                "w8P": w8P,
                "bc": bc,
            }
        )
    return in_maps


def run_on_device(f, g, W1, b1, W2, b2, **spmd_kwargs):
    """Runs the kernel; returns (logits, BassKernelResults)."""
    nc = _get_program()
    in_maps = _prep_inputs(f, g, W1, b1, W2, b2)
    res = run_bass_kernel_spmd(nc, in_maps, list(range(B)), **spmd_kwargs)
    out = np.empty((B, T, U, V), dtype=np.float32)
    inv = np.float32(1.0 / LOUT)
    inv_ls = np.float32(1.0 / LS)
    b2f = b2.astype(np.float32)
    for i in range(B):
        r = res.results[i]
        full = r["outT"].astype(np.float32).reshape(V, U, T) * inv  # s2@W2 part
        mtw = r["mtwP"].transpose(1, 0, 2).reshape(V, T) * inv_ls  # (V,T)
        cuw = r["cuwP"].transpose(1, 0, 2).reshape(V, U) * inv_ls  # (V,U)
        full += mtw[:, None, :]
        full += cuw[:, :, None]
        full += b2f[:, None, None]
        out[i] = full.transpose(2, 1, 0)
    return out, res


def kernel(f, g, W1, b1, W2, b2):
    out, _ = run_on_device(f, g, W1, b1, W2, b2)
    return out


# revision 22
# speedup vs baseline: 1.0598x; 1.0338x over previous
"""RNN-T joint network kernel for Trainium2 (8 NeuronCores, data-parallel over B).

Computes logits = relu(f @ W1f.T + g @ W1g.T + b1) @ W2.T + b2 over the
(B, T, U, ...) broadcast grid without materializing the concat tensor.

v2 strategy (per core, one batch element b) — fp8 DoubleRow main matmul:
  - The dominant cost is h @ W2.T (10.6 GMAC/core).  fp16 runs it at
    ~1 col/cycle/128-chunk on the PE; fp8e4 (e4m3) with
    MatmulPerfMode.DoubleRow contracts 256 K per column-cycle => 2x.
  - Direct e4m3 quantization of h fails the 2e-2 gate (0.044).  Fix:
    subtract a rank-structured mean field before quantizing.  With
    x = pf[t,j] + pg[u,j] (+b1) and q = relu(x):
        s2[j,t,u] = q - mt[j,t] - cu[j,u]
    where mt[j,t] ~ E_u[q] and cu[j,u] ~ E_t[q] - E[q] are the two-way
    ANOVA main effects, computed in closed form on-device:
    pf[.,j]/pg[.,j] are (exactly) Gaussian across t/u, so
    E[relu(z + sigma Z)] = sigma*(phi(zh) + zh*Phi(zh)) = sigma*(0.3989*
    exp(-zh^2/2) + Gelu(zh)) — ScalarE Gelu/Exp + tiny DVE ops.
    s2 has ~3x smaller magnitude than q, so e4m3(s2) passes easily
    (measured end-to-end rel err ~0.012 vs gate 2e-2).
  - s-gen is fully fused per (jc, u):  a = max(pf + (pg-cu), -cu)
    (= relu(x) - cu, one DVE tensor_scalar with two per-partition AP
    scalars), then s8 = e4m3(a - mt) (one tensor_tensor, fp8 out).
  - Scales: lambda_s=16 folded into W1/b1 on host, lambda_w=16 into the
    e4m3 W2 copy => PSUM = 256 * (s2@W2) and the int8 output scale is
    exactly 256: the drain is a bare fp32->int8 cast (ScalarE Identity
    activation for 7 vocab chunks, DVE tensor_copy for 1).
  - The rank images mt@W2 (V x T) and cu@W2 (V x U) are computed on
    device with tiny fp16 matmuls and DMA'd out; the host adds them
    (plus b2) during the existing dequant/transpose pass.
  - Grid u-major: g = u*T + t; spans of 10 u-rows (2000 cols, 4 PSUM
    banks of 500); the single-u tail span (u=100) is processed first to
    prime the pipeline.
"""

import sys

sys.path.insert(0, "/opt/trn_rl_repo")

import ml_dtypes
import numpy as np

from concourse import bacc, bass, tile, mybir
from concourse.bass_utils import run_bass_kernel_spmd

B, T, U = 8, 200, 101
ENC_H, PRED_H, JH, V = 1024, 320, 512, 1024
PRED_P = 384  # PRED_H zero-padded to a multiple of 128
G = U * T  # 20200 grid points per core, u-major: g = u*T + t
GW = U + JH  # g and w1g packed side by side, one DMA
LS = 16.0  # lambda_s: h-side scale, folded into W1/b1 on host
LW = 16.0  # lambda_w: W2 scale for the e4m3 copy
# PSUM = LS*LW*(s2@W2) = 256*(s2@W2); |256*s2@W2| < ~120 so int8 = RNE(psum)
LOUT = LS * LW
INV_PHI = 0.3989422804014327  # 1/sqrt(2*pi)
SC_DRAIN_VCS = (0, 1, 2, 3, 4, 5, 6, 7)  # vocab chunks drained on ScalarE

F32 = mybir.dt.float32
F16 = mybir.dt.float16
F8 = mybir.dt.float8e4
I8 = mybir.dt.int8
AF = mybir.ActivationFunctionType
ALU = mybir.AluOpType
DR = mybir.MatmulPerfMode.DoubleRow

_CACHE = {}


def _build_program():
    nc = bacc.Bacc(None, target_bir_lowering=False)

    fP = nc.declare_dram_parameter("fP", [128, 8, T], F16, isOutput=False)
    gwP = nc.declare_dram_parameter("gwP", [128, 3, GW], F16, isOutput=False)
    w1fP = nc.declare_dram_parameter("w1fP", [128, 8, JH], F16, isOutput=False)
    w2P = nc.declare_dram_parameter("w2P", [128, 4, V], F16, isOutput=False)
    w8P = nc.declare_dram_parameter("w8P", [128, 4, V], F8, isOutput=False)
    bc = nc.declare_dram_parameter("bc", [128, 4], F32, isOutput=False)
    outT = nc.declare_dram_parameter("outT", [V, G], I8, isOutput=True)
    mtwP = nc.declare_dram_parameter("mtwP", [128, 8, T], F32, isOutput=True)
    cuwP = nc.declare_dram_parameter("cuwP", [128, 8, U], F32, isOutput=True)

    with tile.TileContext(nc) as tc:
        with (
            tc.tile_pool(name="const", bufs=1) as const,
            tc.tile_pool(name="hbuf", bufs=4) as hbuf,
            tc.tile_pool(name="obuf", bufs=8) as obuf,
            tc.tile_pool(name="psum", bufs=2, space="PSUM") as psum,
        ):
            # ---- load inputs (small tensors first; HWDGE ring drains FIFO) ----
            gw_sb = const.tile([128, 3, GW], F16, tag="gw_sb")
            for c in range(3):
                nc.sync.dma_start(gw_sb[:, c, :], gwP[:, c, :])
            b_sb = const.tile([128, 4], F32, tag="b_sb")
            nc.sync.dma_start(b_sb[:, :], bc[:, :])
            f_sb = const.tile([128, 8, T], F16, tag="f_sb")
            w1f_sb = const.tile([128, 8, JH], F16, tag="w1f_sb")
            w8_sb = const.tile([128, 4, V], F8, tag="w8_sb")
            w2_sb = const.tile([128, 4, V], F16, tag="w2_sb")
            for q in range(4):
                nc.sync.dma_start(
                    f_sb[:, 2 * q : 2 * q + 2, :], fP[:, 2 * q : 2 * q + 2, :]
                )
                nc.sync.dma_start(
                    w1f_sb[:, 2 * q : 2 * q + 2, :], w1fP[:, 2 * q : 2 * q + 2, :]
                )
                nc.sync.dma_start(w8_sb[:, q, :], w8P[:, q, :])
            for q in range(4):
                nc.sync.dma_start(w2_sb[:, q, :], w2P[:, q, :])

            # ---- first-layer projections (LS-scaled via host-side W1/b1) ----
            pg_ps = psum.tile([128, 4, 512], F32, tag="pt")
            for jc in range(4):
                for c in range(3):
                    nc.tensor.matmul(
                        pg_ps[:, jc, :U],
                        gw_sb[:, c, U + jc * 128 : U + (jc + 1) * 128],
                        gw_sb[:, c, :U],
                        start=(c == 0),
                        stop=(c == 2),
                    )
            pg_sb = const.tile([128, 4, U], F32, tag="pg_sb")
            for jc in range(4):
                nc.vector.tensor_scalar(
                    pg_sb[:, jc, :],
                    pg_ps[:, jc, :U],
                    b_sb[:, jc : jc + 1],
                    None,
                    ALU.add,
                )
            pf_ps = psum.tile([128, 4, 512], F32, tag="pt")
            for hc in range(8):
                for jc in range(4):
                    nc.tensor.matmul(
                        pf_ps[:, jc, :T],
                        w1f_sb[:, hc, jc * 128 : (jc + 1) * 128],
                        f_sb[:, hc, :],
                        start=(hc == 0),
                        stop=(hc == 7),
                    )
            pf16 = const.tile([128, 4, T], F16, tag="pf16")
            for jc in range(4):
                nc.vector.tensor_copy(pf16[:, jc, :], pf_ps[:, jc, :T])

            # ---- per-j moments over t (of pf) and u (of pg) ----
            stat = const.tile([128, 22, 4], F32, tag="stat")
            (
                mu_f, e2_f, msq_f, var_f, sd_f, rs_f, mu_g, e2_g, msq_g, var_g,
                sd_g, rs_g, g0, vs_, sg_, rg_, ag, gg, sqg, eg, tg, grand,
            ) = (stat[:, i, :] for i in range(22))

            pfsq = const.tile([128, 4, T], F32, tag="pfsq")
            nc.vector.tensor_tensor(pfsq[:, :, :], pf16[:, :, :], pf16[:, :, :], ALU.mult)
            nc.vector.tensor_reduce(mu_f[:, :], pf16[:, :, :], mybir.AxisListType.X, ALU.add)
            nc.vector.tensor_reduce(e2_f[:, :], pfsq[:, :, :], mybir.AxisListType.X, ALU.add)
            pgsq = const.tile([128, 4, U], F32, tag="pgsq")
            nc.vector.tensor_tensor(pgsq[:, :, :], pg_sb[:, :, :], pg_sb[:, :, :], ALU.mult)
            nc.vector.tensor_reduce(mu_g[:, :], pg_sb[:, :, :], mybir.AxisListType.X, ALU.add)
            nc.vector.tensor_reduce(e2_g[:, :], pgsq[:, :, :], mybir.AxisListType.X, ALU.add)
            nc.vector.tensor_scalar(mu_f[:, :], mu_f[:, :], 1.0 / T, None, ALU.mult)
            nc.vector.tensor_scalar(e2_f[:, :], e2_f[:, :], 1.0 / T, None, ALU.mult)
            nc.vector.tensor_scalar(mu_g[:, :], mu_g[:, :], 1.0 / U, None, ALU.mult)
            nc.vector.tensor_scalar(e2_g[:, :], e2_g[:, :], 1.0 / U, None, ALU.mult)
            nc.vector.tensor_tensor(msq_f[:, :], mu_f[:, :], mu_f[:, :], ALU.mult)
            nc.vector.tensor_tensor(msq_g[:, :], mu_g[:, :], mu_g[:, :], ALU.mult)
            nc.vector.scalar_tensor_tensor(
                var_f[:, :], e2_f[:, :], 1e-5, msq_f[:, :], ALU.add, ALU.subtract
            )
            nc.vector.scalar_tensor_tensor(
                var_g[:, :], e2_g[:, :], 1e-5, msq_g[:, :], ALU.add, ALU.subtract
            )
            nc.scalar.activation(sd_f[:, :], var_f[:, :], AF.Sqrt)
            nc.vector.reciprocal(rs_f[:, :], sd_f[:, :])
            nc.scalar.activation(sd_g[:, :], var_g[:, :], AF.Sqrt)
            nc.vector.reciprocal(rs_g[:, :], sd_g[:, :])
            nc.vector.tensor_tensor(g0[:, :], mu_f[:, :], mu_g[:, :], ALU.add)
            nc.vector.tensor_tensor(vs_[:, :], var_f[:, :], var_g[:, :], ALU.add)
            nc.scalar.activation(sg_[:, :], vs_[:, :], AF.Sqrt)
            nc.vector.reciprocal(rg_[:, :], sg_[:, :])

            # ---- analytic mean fields: psi(zh) = Gelu(zh) + phi(zh) ----
            # (Gelu/Exp calls batched so ScalarE loads each ACT table once.)
            # mt[j,t] = sd_g * psi((pf + mu_g)/sd_g)   (E_u[relu(x)])
            zh = const.tile([128, 4, T], F32, tag="zh")
            for jc in range(4):
                nc.vector.tensor_scalar(
                    zh[:, jc, :],
                    pf16[:, jc, :],
                    mu_g[:, jc : jc + 1],
                    rs_g[:, jc : jc + 1],
                    ALU.add,
                    ALU.mult,
                )
            # yh for cu[j,u] = sd_f * psi((pg + mu_f)/sd_f) - grand
            yh = const.tile([128, 4, U], F32, tag="yh")
            for jc in range(4):
                nc.vector.tensor_scalar(
                    yh[:, jc, :],
                    pg_sb[:, jc, :],
                    mu_f[:, jc : jc + 1],
                    rs_f[:, jc : jc + 1],
                    ALU.add,
                    ALU.mult,
                )
            nc.vector.tensor_tensor(ag[:, :], g0[:, :], rg_[:, :], ALU.mult)
            g1t = const.tile([128, 4, T], F32, tag="g1t")
            g1u = const.tile([128, 4, U], F32, tag="g1u")
            nc.scalar.activation(g1t[:, :, :], zh[:, :, :], AF.Gelu)
            nc.scalar.activation(g1u[:, :, :], yh[:, :, :], AF.Gelu)
            nc.scalar.activation(gg[:, :], ag[:, :], AF.Gelu)
            sqt = pfsq  # reuse
            nc.vector.tensor_tensor(sqt[:, :, :], zh[:, :, :], zh[:, :, :], ALU.mult)
            squ = pgsq  # reuse
            nc.vector.tensor_tensor(squ[:, :, :], yh[:, :, :], yh[:, :, :], ALU.mult)
            nc.vector.tensor_tensor(sqg[:, :], ag[:, :], ag[:, :], ALU.mult)
            e1t = const.tile([128, 4, T], F32, tag="e1t")
            e1u = const.tile([128, 4, U], F32, tag="e1u")
            nc.scalar.activation(e1t[:, :, :], sqt[:, :, :], AF.Exp, scale=-0.5)
            nc.scalar.activation(e1u[:, :, :], squ[:, :, :], AF.Exp, scale=-0.5)
            nc.scalar.activation(eg[:, :], sqg[:, :], AF.Exp, scale=-0.5)
            tmpt = zh  # reuse
            nc.vector.scalar_tensor_tensor(
                tmpt[:, :, :], e1t[:, :, :], INV_PHI, g1t[:, :, :], ALU.mult, ALU.add
            )
            mt16 = const.tile([128, 4, T], F16, tag="mt16")
            for jc in range(4):
                nc.vector.tensor_scalar(
                    mt16[:, jc, :], tmpt[:, jc, :], sd_g[:, jc : jc + 1], None, ALU.mult
                )
            tmpu = yh  # reuse
            nc.vector.scalar_tensor_tensor(
                tmpu[:, :, :], e1u[:, :, :], INV_PHI, g1u[:, :, :], ALU.mult, ALU.add
            )
            nc.vector.scalar_tensor_tensor(
                tg[:, :], eg[:, :], INV_PHI, gg[:, :], ALU.mult, ALU.add
            )
            nc.vector.tensor_tensor(grand[:, :], tg[:, :], sg_[:, :], ALU.mult)

            cu16 = const.tile([128, 4, U], F16, tag="cu16")
            for jc in range(4):
                nc.vector.tensor_scalar(
                    cu16[:, jc, :],
                    tmpu[:, jc, :],
                    sd_f[:, jc : jc + 1],
                    grand[:, jc : jc + 1],
                    ALU.mult,
                    ALU.subtract,
                )
            cun32 = const.tile([128, 4, U], F32, tag="cun32")
            pgc = const.tile([128, 4, U], F32, tag="pgc")
            for jc in range(4):
                nc.vector.tensor_scalar(
                    cun32[:, jc, :], cu16[:, jc, :], -1.0, None, ALU.mult
                )
                nc.vector.tensor_tensor(
                    pgc[:, jc, :], pg_sb[:, jc, :], cu16[:, jc, :], ALU.subtract
                )

            # ---- rank images mt@W2, cu@W2 (fp16 matmuls; host adds them) ----
            rk1 = psum.tile([128, 4, 512], F32, tag="pt")
            for vc in range(8):
                sl = rk1[:, vc // 2, (vc % 2) * 256 : (vc % 2) * 256 + T]
                for jc in range(4):
                    nc.tensor.matmul(
                        sl,
                        w2_sb[:, jc, vc * 128 : (vc + 1) * 128],
                        mt16[:, jc, :],
                        start=(jc == 0),
                        stop=(jc == 3),
                    )
            mtw_sb = const.tile([128, 8, T], F32, tag="mtw_sb")
            for vc in range(8):
                nc.scalar.activation(
                    mtw_sb[:, vc, :],
                    rk1[:, vc // 2, (vc % 2) * 256 : (vc % 2) * 256 + T],
                    AF.Identity,
                )
            nc.sync.dma_start(mtwP[:, :, :], mtw_sb[:, :, :])
            rk2 = psum.tile([128, 4, 512], F32, tag="pt")
            for vc in range(8):
                sl = rk2[:, vc // 2, (vc % 2) * 256 : (vc % 2) * 256 + U]
                for jc in range(4):
                    nc.tensor.matmul(
                        sl,
                        w2_sb[:, jc, vc * 128 : (vc + 1) * 128],
                        cu16[:, jc, :],
                        start=(jc == 0),
                        stop=(jc == 3),
                    )
            cuw_sb = const.tile([128, 8, U], F32, tag="cuw_sb")
            for vc in range(8):
                nc.scalar.activation(
                    cuw_sb[:, vc, :],
                    rk2[:, vc // 2, (vc % 2) * 256 : (vc % 2) * 256 + U],
                    AF.Identity,
                )
            nc.sync.dma_start(cuwP[:, :, :], cuw_sb[:, :, :])

            # ---- main loop: spans of 10 u-rows (2000 cols); tail (u=100) first
            spans = [(100, 1)] + [(10 * s, 10) for s in range(10)]

            def emit_hgen(si):
                """s-gen for span si; returns the two fp8 h tile-pairs."""
                u0, nu = spans[si]
                glen = nu * T
                h0 = hbuf.tile([128, 2, 2000], F8, tag="h0")
                h1 = hbuf.tile([128, 2, 2000], F8, tag="h1")
                for jc in range(4):
                    hp = h0 if jc < 2 else h1
                    half = jc % 2
                    asp = hbuf.tile([128, 10, T], F16, tag=f"a{jc}")
                    for ui in range(nu):
                        u = u0 + ui
                        # a = max(pf + (pg-cu), -cu) = relu(pf+pg) - cu
                        nc.vector.tensor_scalar(
                            asp[:, ui, :],
                            pf16[:, jc, :],
                            pgc[:, jc, u : u + 1],
                            cun32[:, jc, u : u + 1],
                            ALU.add,
                            ALU.max,
                        )
                    # s8 = e4m3(a - mt): one span-wide op, mt broadcast over u
                    nc.vector.tensor_tensor(
                        hp[:, half, :glen].rearrange("p (a b) -> p a b", b=T),
                        asp[:, :nu, :],
                        mt16[:, jc : jc + 1, :].broadcast_to([128, nu, T]),
                        ALU.subtract,
                    )
                return h0, h1

            # software pipeline: h-gen for span si+1 is emitted before the
            # matmuls/drains of span si, so the producer-side DVE/ScalarE ops
            # sit ahead of span si's drains in each engine's FIFO queue.
            cur = emit_hgen(0)
            for si, (u0, nu) in enumerate(spans):
                glen = nu * T
                last = si == len(spans) - 1
                h0, h1 = cur
                if not last:
                    cur = emit_hgen(si + 1)
                nb = (glen + 499) // 500
                for vc in range(8):
                    pt = psum.tile([128, 4, 512], F32, tag="pt")
                    for p in (0, 1):
                        hp = h0 if p == 0 else h1
                        for b in range(nb):
                            blen = min(500, glen - b * 500)
                            nc.tensor.matmul(
                                pt[:, b, :blen],
                                w8_sb[:, 2 * p : 2 * p + 2, vc * 128 : (vc + 1) * 128],
                                hp[:, :, b * 500 : b * 500 + blen],
                                start=(p == 0),
                                stop=(p == 1),
                                perf_mode=DR,
                            )
                    ob = obuf.tile([128, 4, 500], I8, tag="ob")
                    g0c = u0 * T
                    if last and vc == 7:
                        # final drain: split so the first half's DMA overlaps
                        # the second half's drain
                        for hb in (0, 2):
                            nc.scalar.activation(
                                ob[:, hb : hb + 2, :], pt[:, hb : hb + 2, :500], AF.Identity
                            )
                            nc.scalar.dma_start(
                                outT[
                                    vc * 128 : (vc + 1) * 128,
                                    g0c + hb * 500 : g0c + (hb + 2) * 500,
                                ],
                                ob[:, hb : hb + 2, :],
                            )
                        continue
                    if nu == 1:
                        src, dst = pt[:, 0, :glen], ob[:, 0, :glen]
                    else:
                        src, dst = pt[:, :, :500], ob[:, :, :]
                    if (vc in SC_DRAIN_VCS) and not (last and vc % 2 == 1):
                        nc.scalar.activation(dst, src, AF.Identity)
                    else:
                        nc.vector.tensor_copy(dst, src)
                    deng = nc.sync if vc % 2 == 0 else nc.scalar
                    deng.dma_start(
                        outT[vc * 128 : (vc + 1) * 128, g0c : g0c + glen], dst
                    )

    nc.compile()
    return nc


def _get_program():
    if "nc" not in _CACHE:
        _CACHE["nc"] = _build_program()
    return _CACHE["nc"]


def _pmajor(mT, nchunks):
    """[nchunks*128, free] -> [128, nchunks, free] partition-major layout."""
    free = mT.shape[1]
    return np.ascontiguousarray(mT.reshape(nchunks, 128, free).transpose(1, 0, 2))


def _prep_inputs(f, g, W1, b1, W2, b2):
    f16 = np.float16
    W1fT = (W1[:, :ENC_H].T * LS).astype(f16)  # (1024, 512), LS-scaled
    W1gT = np.zeros((PRED_P, JH), dtype=f16)
    W1gT[:PRED_H] = (W1[:, ENC_H:].T * LS).astype(f16)
    W2T = W2.T.astype(f16)  # (512, 1024) unscaled, for the rank matmuls
    w1fP = _pmajor(W1fT, 8)
    w1gP = _pmajor(W1gT, 3)
    w2P = _pmajor(W2T, 4)
    w8P = _pmajor((W2.T * LW).astype(np.float32).astype(ml_dtypes.float8_e4m3), 4)
    bc = np.ascontiguousarray((LS * b1).reshape(4, 128).T.astype(np.float32))
    in_maps = []
    for i in range(B):
        gTp = np.zeros((PRED_P, U), dtype=f16)
        gTp[:PRED_H] = g[i].T.astype(f16)
        gwP = np.empty((128, 3, GW), dtype=f16)
        gwP[:, :, :U] = _pmajor(gTp, 3)
        gwP[:, :, U:] = w1gP
        in_maps.append(
            {
                "fP": _pmajor(f[i].T.astype(f16), 8),
                "gwP": gwP,
                "w1fP": w1fP,
                "w2P": w2P,
                "w8P": w8P,
                "bc": bc,
            }
        )
    return in_maps


def run_on_device(f, g, W1, b1, W2, b2, **spmd_kwargs):
    """Runs the kernel; returns (logits, BassKernelResults)."""
    nc = _get_program()
    in_maps = _prep_inputs(f, g, W1, b1, W2, b2)
    res = run_bass_kernel_spmd(nc, in_maps, list(range(B)), **spmd_kwargs)
    out = np.empty((B, T, U, V), dtype=np.float32)
    inv = np.float32(1.0 / LOUT)
    inv_ls = np.float32(1.0 / LS)
    b2f = b2.astype(np.float32)
    for i in range(B):
        r = res.results[i]
        full = r["outT"].astype(np.float32).reshape(V, U, T) * inv  # s2@W2 part
        mtw = r["mtwP"].transpose(1, 0, 2).reshape(V, T) * inv_ls  # (V,T)
        cuw = r["cuwP"].transpose(1, 0, 2).reshape(V, U) * inv_ls  # (V,U)
        full += mtw[:, None, :]
        full += cuw[:, :, None]
        full += b2f[:, None, None]
        out[i] = full.transpose(2, 1, 0)
    return out, res


def kernel(f, g, W1, b1, W2, b2):
    out, _ = run_on_device(f, g, W1, b1, W2, b2)
    return out
